# revision 1
# baseline (speedup 1.0000x reference)
"""DeepSATConv GNN message-passing kernel for 8 Trainium2 NeuronCores.

Math note: the reference computes a per-channel segment-softmax over
msg = self_h[src] + neib_h[dst].  Within a dst-segment, neib_h[dst] (and
b_self, b_nb) are constant per channel, so they cancel in the softmax.
Hence alpha = segsoftmax(h[src] @ W_self.T) exactly, and
out[n] = segsum(e * h[src]) / segsum(e)  with e = exp((h @ W_self.T)[src]),
falling back to h[n] for zero-in-degree nodes.  W_nb / b_nb / b_self do
not affect the output at all.

Sharding: nodes are split across the 8 cores (2500 each); edges are
partitioned by destination node so segment reductions stay core-local;
h is replicated (the "halo gather" degenerates to replication).

Per core the kernel
  A) computes Z = h @ [W_self.T | I] = [self_h | h] for all nodes into
     core-local HBM (replicated compute; cheaper than collectives, and
     packing h alongside self_h lets one dma_gather descriptor fetch
     both operands per edge — SWDGE descriptor generation on the Q7 is
     the dominant cost of gathers),
  B) for each 128-node tile, dma_gathers Z[src] for the tile's
     (dst-sorted, padded) edge list, then for each 128-edge chunk
     builds a one-hot selector S[e, n] = (dst_local[e] == n) on the DVE
     and accumulates  [denom | numer] = S.T @ [exp(sh) | exp(sh) * hs]
     into a PSUM bank over all chunks of the tile,
  C) finalizes out = numer / max(denom, tiny), with copy_predicated
     restoring h for empty nodes, and writes the tile to HBM.
"""

import os
import numpy as np

N_NODES = 20000
N_EDGES = 320000
D = 256
CORES = 8
NPC = N_NODES // CORES          # 2500 nodes per core
NT = (NPC + 127) // 128         # 20 node tiles per core
NROWS = NT * 128                # 2560 padded rows per core
NT_ALL = 160                    # phase-A tiles (two 80-tile Z blocks)
NPAD = NT_ALL * 128             # 20480
NPB = 2                         # Z source blocks (phase A/B overlap)
NBH = NPAD // NPB               # rows per Z block
BB = 6                          # chunks per exp/mult batch

# float32r runs the selector matmul at 4x the fp32 rate but rounds
# operands to ~tf32 precision (~8e-4 output error vs ~3e-5 for fp32).
USE_F32R = os.environ.get("GNN_F32R", "0") == "1"

_cache = {}


def _build(caps):
    import concourse.bacc as bacc
    import concourse.mybir as mybir
    from concourse.tile import TileContext

    nc = bacc.Bacc("TRN2")
    f32 = mybir.dt.float32
    mm_dt = mybir.dt.float32r if USE_F32R else f32

    bf16 = mybir.dt.bfloat16
    NCH = sum(sum(r) for r in caps)     # total chunks across tiles/blocks
    NIX = 128 * NCH                     # total gathered edge slots
    hT_d = nc.dram_tensor("hT", [128, 2, 2, NPAD], bf16, kind="ExternalInput")
    WI_d = nc.dram_tensor("WI", [128, 2, 2, 2 * D], bf16, kind="ExternalInput")
    idx_d = nc.dram_tensor("idx", [128, NIX // 16], mybir.dt.int16, kind="ExternalInput")
    S_d = nc.dram_tensor("S", [128, NCH, 128], f32, kind="ExternalInput")
    hown_d = nc.dram_tensor("hown", [NROWS, D], f32, kind="ExternalInput")
    out_d = nc.dram_tensor("out", [NROWS, D], f32, kind="ExternalOutput")

    CMAX = max(a + b for a, b in caps)
    with TileContext(nc) as tc:
        with (
            tc.tile_pool(name="const", bufs=1) as constp,
            tc.tile_pool(name="pha", bufs=3) as pha,
            tc.tile_pool(name="gat", bufs=2) as gat,
            tc.tile_pool(name="wrk", bufs=6) as wrk,
            tc.tile_pool(name="fin", bufs=2) as fin,
            tc.tile_pool(name="psa", bufs=2, space="PSUM") as psa,
            tc.tile_pool(name="psb", bufs=3, space="PSUM") as psb,
            tc.tile_pool(name="dram", bufs=1, space="DRAM") as dramp,
        ):
            z_blk = []
            for s_ in range(NPB):
                zb = dramp.tile([NBH, 2 * D], f32, tag=f"zblk{s_}")
                z_blk.append(zb)

            # ---- phase A: Z = h @ [W_self.T | I] = [self_h | h], all nodes ----
            # bf16 hi/lo split: h = hi + lo, W.T columns split likewise into
            # WI_hi = [W_hi.T | I], WI_lo = [W_lo.T | 0]; three bf16 products
            # hi@WI_hi + hi@WI_lo + lo@WI_hi reproduce fp32 to ~1e-5.
            WI_sb = constp.tile([128, 2, 2, 2 * D], bf16)
            nc.sync.dma_start(WI_sb[:, :, :, :], WI_d[:, :, :, :])
            for i in range(NT_ALL):
                hT_sb = pha.tile([128, 2, 2, 128], bf16, tag="hT")
                nc.sync.dma_start(hT_sb[:, :, :, :], hT_d[:, :, :, i * 128:(i + 1) * 128])
                ps = psa.tile([128, 2 * D], f32, tag="ps")
                nmm = 0
                for hw, ww in ((0, 0), (0, 1), (1, 0)):
                    for kb in range(2):
                        nc.tensor.matmul(
                            ps[:, :], hT_sb[:, hw, kb, :], WI_sb[:, ww, kb, :],
                            start=(nmm == 0), stop=(nmm == 5),
                        )
                        nmm += 1
                z_sb = pha.tile([128, 2 * D], f32, tag="zs")
                nc.scalar.copy(z_sb[:, :], ps[:, :])
                blk, row = divmod(i * 128, NBH)
                nc.sync.dma_start(z_blk[blk][row:row + 128, :], z_sb[:, :])

            # ---- constants ----
            idx_sb = constp.tile([128, NIX // 16], mybir.dt.int16)
            nc.sync.dma_start(idx_sb[:, :], idx_d[:, :])

            # ---- phase B: per node-tile segment softmax ----
            chunk_off = 0   # global chunk counter (indexes idx/S/dstl layout)
            for t in range(NT):
                zx_t = gat.tile([128, CMAX, 2 * D], f32, tag="zx")
                C_t = caps[t][0] + caps[t][1]
                zoff = 0
                for s_ in range(NPB):
                    Cs = caps[t][s_]
                    if Cs == 0:
                        continue
                    CAPs = 128 * Cs
                    io = (chunk_off + zoff) * 8
                    nc.gpsimd.dma_gather(
                        zx_t[:, zoff:zoff + Cs, :], z_blk[s_][:, :],
                        idx_sb[:, io:io + 8 * Cs], CAPs, CAPs, 2 * D,
                        single_packet=False,
                    )
                    zoff += Cs
                acc = psb.tile([128, 2 * D], f32, tag="acc")
                for g in range((C_t + BB - 1) // BB):
                    b = min(BB, C_t - g * BB)
                    eX = wrk.tile([128, BB, 2 * D], mm_dt, tag="eX")
                    Sg = wrk.tile([128, BB, 128], f32, tag="Sg")
                    so = chunk_off + g * BB
                    nc.sync.dma_start(Sg[:, 0:b, :], S_d[:, so:so + b, :])
                    nc.scalar.activation(
                        eX[:, 0:b, 0:D], zx_t[:, g * BB:g * BB + b, 0:D],
                        mybir.ActivationFunctionType.Exp,
                    )
                    nc.vector.tensor_tensor(
                        eX[:, 0:b, D:2 * D], eX[:, 0:b, 0:D],
                        zx_t[:, g * BB:g * BB + b, D:2 * D],
                        mybir.AluOpType.mult,
                    )
                    for j in range(b):
                        k = g * BB + j
                        nc.tensor.matmul(
                            acc[:, :], Sg[:, j, :], eX[:, j, :],
                            start=(k == 0), stop=(k == C_t - 1),
                        )
                chunk_off += C_t

                # ---- finalize tile ----
                accs = fin.tile([128, 2 * D], f32, tag="accs")
                nc.scalar.copy(accs[:, :], acc[:, :])
                dmax = fin.tile([128, D], f32, tag="dmax")
                nc.vector.tensor_scalar(
                    dmax[:, :], accs[:, 0:D], 1e-37, None, mybir.AluOpType.max
                )
                rec = fin.tile([128, D], f32, tag="rec")
                nc.vector.reciprocal(rec[:, :], dmax[:, :])
                res = fin.tile([128, D], f32, tag="res")
                nc.vector.tensor_tensor(
                    res[:, :], accs[:, D:2 * D], rec[:, :], mybir.AluOpType.mult
                )
                mask = fin.tile([128, D], mybir.dt.uint8, tag="mask")
                nc.vector.tensor_scalar(
                    mask[:, :], accs[:, 0:D], 0.0, None, mybir.AluOpType.is_equal
                )
                hown_sb = fin.tile([128, D], f32, tag="hown")
                nc.sync.dma_start(hown_sb[:, :], hown_d[t * 128:(t + 1) * 128, :])
                nc.vector.copy_predicated(res[:, :], mask[:, :], hown_sb[:, :])
                nc.sync.dma_start(out_d[t * 128:(t + 1) * 128, :], res[:, :])
    nc.compile()
    return nc


def _wrap_idx(ix):
    # dma_gather index layout: logical index i lands at output
    # [partition i%128, slot i//128]; the SBUF index tile stores it at
    # [i%16, 8*(i//128) + (i%128)//16], replicated over the 8 Q7 cores.
    w = ix.astype(np.int16).reshape(-1, 8, 16).transpose(2, 0, 1).reshape(16, -1)
    return np.tile(w, (8, 1))


def kernel(h, W_nb, b_nb, W_self, b_self, src, dst):
    from concourse.bass_utils import run_bass_kernel_spmd

    h = np.ascontiguousarray(np.asarray(h, dtype=np.float32))
    W = np.asarray(W_self, dtype=np.float32)
    src = np.asarray(src, dtype=np.int64)
    dst = np.asarray(dst, dtype=np.int64)

    order = np.argsort(dst, kind="stable")
    src_s = src[order]
    dst_s = dst[order]

    # per-(core, tile) edge ranges; tiles are 128 consecutive owned nodes
    tile_base = []
    for c in range(CORES):
        for t in range(NT):
            tile_base.append(c * NPC + t * 128)
    bounds_lo = np.searchsorted(dst_s, np.array(tile_base), side="left")
    hi_nodes = [min(b + 128, (b // NPC + 1) * NPC) for b in tile_base]
    bounds_hi = np.searchsorted(dst_s, np.array(hi_nodes), side="left")

    # split each tile's edges by src block; caps shared across cores (SPMD)
    per_ct = {}
    cnt = np.zeros((CORES, NT, NPB), dtype=np.int64)
    for c in range(CORES):
        for t in range(NT):
            i = c * NT + t
            lo, hi = int(bounds_lo[i]), int(bounds_hi[i])
            blk = src_s[lo:hi] // NBH
            for s_ in range(NPB):
                sel = np.nonzero(blk == s_)[0]
                per_ct[(c, t, s_)] = (src_s[lo:hi][sel], dst_s[lo:hi][sel] - tile_base[i])
                cnt[c, t, s_] = len(sel)
    caps = [
        [int((cnt[:, t, s_].max() + 127) // 128) for s_ in range(NPB)]
        for t in range(NT)
    ]
    assert max(a + b for a, b in caps) <= 36, f"edge distribution too skewed: {caps}"
    NCH = sum(sum(r) for r in caps)

    # host-side layout prep: bf16 hi/lo split of h and W for phase A
    import ml_dtypes
    bf = ml_dtypes.bfloat16
    h_hi = h.astype(bf)
    h_lo = (h - h_hi.astype(np.float32)).astype(bf)
    W_hi = W.astype(bf)
    W_lo = (W - W_hi.astype(np.float32)).astype(bf)

    hT = np.zeros((2, D, NPAD), dtype=bf)
    hT[0, :, :N_NODES] = h_hi.T
    hT[1, :, :N_NODES] = h_lo.T
    hT = np.ascontiguousarray(
        hT.reshape(2, 2, 128, NPAD).transpose(2, 0, 1, 3)
    )
    WI = np.zeros((2, D, 2 * D), dtype=bf)
    WI[0, :, :D] = W_hi.T
    WI[1, :, :D] = W_lo.T
    WI[0, np.arange(D), D + np.arange(D)] = bf(1.0)
    WI = np.ascontiguousarray(
        WI.reshape(2, 2, 128, 2 * D).transpose(2, 0, 1, 3)
    )

    in_maps = []
    for c in range(CORES):
        idx_parts = []
        S_all = np.zeros((128, NCH, 128), dtype=np.float32)
        coff = 0
        for t in range(NT):
            for s_ in range(NPB):
                Cs = caps[t][s_]
                if Cs == 0:
                    continue
                CAPs = 128 * Cs
                ss, dl_real = per_ct[(c, t, s_)]
                n = len(ss)
                spad = np.zeros(CAPs, dtype=np.int64)
                spad[:n] = ss - s_ * NBH      # block-local row index
                dl = np.full(CAPs, -1, dtype=np.int64)
                dl[:n] = dl_real
                idx_parts.append(_wrap_idx(spad))
                ei = np.nonzero(dl >= 0)[0]
                S_all[ei % 128, coff + ei // 128, dl[ei]] = 1.0
                coff += Cs
        hown = np.zeros((NROWS, D), dtype=np.float32)
        hown[:NPC] = h[c * NPC:(c + 1) * NPC]
        in_maps.append({
            "hT": hT,
            "WI": WI,
            "idx": np.ascontiguousarray(np.concatenate(idx_parts, axis=1)),
            "S": S_all,
            "hown": hown,
        })

    key = tuple(tuple(r) for r in caps)
    if key not in _cache:
        _cache[key] = _build(caps)
    nc = _cache[key]

    res = run_bass_kernel_spmd(nc, in_maps, core_ids=list(range(CORES)))
    out = np.concatenate(
        [res.results[c]["out"][:NPC] for c in range(CORES)], axis=0
    )
    return out.astype(np.float32)



# revision 4
# speedup vs baseline: 1.5680x; 1.5680x over previous
"""DeepSATConv GNN message-passing kernel for 8 Trainium2 NeuronCores.

Math note: the reference computes a per-channel segment-softmax over
msg = self_h[src] + neib_h[dst].  Within a dst-segment, neib_h[dst] (and
b_self, b_nb) are constant per channel, so they cancel in the softmax.
Hence alpha = segsoftmax(h[src] @ W_self.T) exactly, and
out[n] = segsum(e * h[src]) / segsum(e)  with e = exp((h @ W_self.T)[src]),
falling back to h[n] for zero-in-degree nodes.  W_nb / b_nb / b_self do
not affect the output at all.

Because e and e*h are pure per-NODE quantities, phase A precomputes
Z = [E | Y] = [exp(sh) | exp(sh)*h] in fp16 for all nodes (per-node work,
2.2x less than per-edge).  Phase B is then just a 1KB-per-edge dma_gather
of Z[src] plus one-hot selector matmuls in fp16 (1 PE cycle/row vs 4 for
fp32) accumulating [denom | numer] per 128-node tile in PSUM:
  acc = sum_chunks S_j.T @ Zx_j,  S_j[e, n] = (dst_local[e] == n)
with S_j built on the otherwise-idle DVE from an iota/is_equal compare.
Z lives in DRAM in a partition-interleaved layout (node n -> row
(n%128)*80 + (n//128)%80 of its half) so phase A can write 4 node-tiles
per DMA with 4KB contiguous per partition.

Sharding: nodes are split across the 8 cores (2500 each); edges are
partitioned by destination node so segment reductions stay core-local;
phase A is replicated (cheaper than collectives at this size).
"""

import numpy as np

N_NODES = 20000
N_EDGES = 320000
D = 256
CORES = 8
NPC = N_NODES // CORES          # 2500 nodes per core
NT = (NPC + 127) // 128         # 20 node tiles per core
NROWS = NT * 128                # 2560 padded rows per core
NT_ALL = 160                    # phase-A tiles over all nodes
NPAD = NT_ALL * 128             # 20480
NPB = 2                         # Z source blocks (phase A/B overlap)
TPB = NT_ALL // NPB             # 80 tiles per Z block
NBN = 128 * TPB                 # 10240 nodes per Z block
SLAB = 8                        # phase-A hT tiles per DMA load
WB = 4                          # Z tiles buffered per DMA write

_cache = {}


def _build(caps):
    import concourse.bacc as bacc
    import concourse.mybir as mybir
    from concourse.tile import TileContext

    nc = bacc.Bacc("TRN2")
    f32 = mybir.dt.float32
    f16 = mybir.dt.float16
    bf16 = mybir.dt.bfloat16

    NCH = sum(sum(r) for r in caps)     # total chunks across tiles/blocks
    NIX = 128 * NCH                     # total gathered edge slots
    hT_d = nc.dram_tensor(
        "hT", [128, NT_ALL // SLAB, SLAB, 2, 128], bf16, kind="ExternalInput"
    )
    WI_d = nc.dram_tensor("WI", [128, 2, 2 * D], bf16, kind="ExternalInput")
    iota_d = nc.dram_tensor("iota", [128, 128], f32, kind="ExternalInput")
    dstl_d = nc.dram_tensor("dstl", [128, NCH], f32, kind="ExternalInput")
    idx_d = nc.dram_tensor("idx", [128, NIX // 16], mybir.dt.int16, kind="ExternalInput")
    hown_d = nc.dram_tensor("hown", [NROWS, D], f32, kind="ExternalInput")
    out_d = nc.dram_tensor("out", [NROWS, D], f32, kind="ExternalOutput")

    CMAX = max(a + b for a, b in caps)
    with TileContext(nc) as tc:
        with (
            tc.tile_pool(name="const", bufs=1) as constp,
            tc.tile_pool(name="pha", bufs=3) as pha,
            tc.tile_pool(name="phz", bufs=3) as phz,
            tc.tile_pool(name="gat", bufs=3) as gat,
            tc.tile_pool(name="wrk", bufs=8) as wrk,
            tc.tile_pool(name="fin", bufs=2) as fin,
            tc.tile_pool(name="psa", bufs=3, space="PSUM") as psa,
            tc.tile_pool(name="psb", bufs=3, space="PSUM") as psb,
            tc.tile_pool(name="dram", bufs=1, space="DRAM") as dramp,
        ):
            z_blk = [
                dramp.tile(
                    [128, TPB, 2 * D], f16, tag=f"zblk{s_}", name=f"zblk{s_}"
                )
                for s_ in range(NPB)
            ]

            # ---- constants ----
            WI_sb = constp.tile([128, 2, 2 * D], bf16)
            nc.sync.dma_start(WI_sb[:, :, :], WI_d[:, :, :])
            iota_sb = constp.tile([128, 128], f32)
            nc.sync.dma_start(iota_sb[:, :], iota_d[:, :])
            dstl_sb = constp.tile([128, NCH], f32)
            nc.sync.dma_start(dstl_sb[:, :], dstl_d[:, :])
            idx_sb = constp.tile([128, NIX // 16], mybir.dt.int16)
            nc.sync.dma_start(idx_sb[:, :], idx_d[:, :])

            # ---- phase A: Z = [exp(sh) | exp(sh)*h], sh = h @ W_self.T ----
            # one bf16 product (~0.4% on exp, averaged down by the softmax);
            # h reaches PSUM fp32 via the [W.T | I] identity columns.
            zq = None
            for g in range(NT_ALL // SLAB):
                hT_sb = pha.tile([128, SLAB, 2, 128], bf16, tag="hT")
                nc.sync.dma_start(hT_sb[:, :, :, :], hT_d[:, g, :, :, :])
                for j in range(SLAB):
                    i = g * SLAB + j
                    ps = psa.tile([128, 2 * D], f32, tag="ps")
                    for kb in range(2):
                        nc.tensor.matmul(
                            ps[:, :], hT_sb[:, j, kb, :], WI_sb[:, kb, :],
                            start=(kb == 0), stop=(kb == 1),
                        )
                    if i % WB == 0:
                        zq = phz.tile([128, WB, 2 * D], f16, tag="zq")
                    w = i % WB
                    e32 = wrk.tile([128, D], f32, tag="e32")
                    nc.scalar.activation(
                        e32[:, :], ps[:, 0:D], mybir.ActivationFunctionType.Exp
                    )
                    nc.vector.tensor_tensor(
                        zq[:, w, D:2 * D], e32[:, :], ps[:, D:2 * D],
                        mybir.AluOpType.mult,
                    )
                    nc.scalar.copy(zq[:, w, 0:D], e32[:, :])
                    if w == WB - 1:
                        s_, tb = divmod(i, TPB)
                        nc.sync.dma_start(
                            z_blk[s_][:, tb - (WB - 1):tb + 1, :], zq[:, :, :]
                        )

            # ---- phase B: per node-tile gather + segment softmax-sum ----
            chunk_off = 0   # global chunk counter (indexes idx/dstl layout)
            for t in range(NT):
                zx_t = gat.tile([128, CMAX, 2 * D], f16, tag="zx")
                C_t = caps[t][0] + caps[t][1]
                zoff = 0
                for s_ in range(NPB):
                    Cs = caps[t][s_]
                    if Cs == 0:
                        continue
                    CAPs = 128 * Cs
                    io = (chunk_off + zoff) * 8
                    nc.gpsimd.dma_gather(
                        zx_t[:, zoff:zoff + Cs, :],
                        z_blk[s_][:, :, :].flatten_outer_dims(),
                        idx_sb[:, io:io + 8 * Cs], CAPs, CAPs, 2 * D,
                        single_packet=False,
                    )
                    zoff += Cs
                acc = psb.tile([128, 2 * D], f32, tag="acc")
                for j in range(C_t):
                    Sg = wrk.tile([128, 128], f16, tag="Sg")
                    k = chunk_off + j
                    nc.vector.tensor_scalar(
                        Sg[:, :], iota_sb[:, :], dstl_sb[:, k:k + 1], None,
                        mybir.AluOpType.is_equal,
                    )
                    nc.tensor.matmul(
                        acc[:, :], Sg[:, :], zx_t[:, j, :],
                        start=(j == 0), stop=(j == C_t - 1),
                    )
                chunk_off += C_t

                # ---- finalize tile: out = numer/denom, h for empty nodes ----
                dmax = fin.tile([128, D], f32, tag="dmax")
                nc.vector.tensor_scalar(
                    dmax[:, :], acc[:, 0:D], 1e-37, None, mybir.AluOpType.max
                )
                rec = fin.tile([128, D], f32, tag="rec")
                nc.vector.reciprocal(rec[:, :], dmax[:, :])
                res = fin.tile([128, D], f32, tag="res")
                nc.vector.tensor_tensor(
                    res[:, :], acc[:, D:2 * D], rec[:, :], mybir.AluOpType.mult
                )
                mask = fin.tile([128, D], mybir.dt.uint8, tag="mask")
                nc.vector.tensor_scalar(
                    mask[:, :], acc[:, 0:D], 0.0, None, mybir.AluOpType.is_equal
                )
                hown_sb = fin.tile([128, D], f32, tag="hown")
                nc.sync.dma_start(hown_sb[:, :], hown_d[t * 128:(t + 1) * 128, :])
                nc.vector.copy_predicated(res[:, :], mask[:, :], hown_sb[:, :])
                nc.sync.dma_start(out_d[t * 128:(t + 1) * 128, :], res[:, :])
    nc.compile()
    return nc


def _wrap_idx(ix):
    # dma_gather index layout: logical index i lands at output
    # [partition i%128, slot i//128]; the SBUF index tile stores it at
    # [i%16, 8*(i//128) + (i%128)//16], replicated over the 8 Q7 cores.
    w = ix.astype(np.int16).reshape(-1, 8, 16).transpose(2, 0, 1).reshape(16, -1)
    return np.tile(w, (8, 1))


def kernel(h, W_nb, b_nb, W_self, b_self, src, dst):
    from concourse.bass_utils import run_bass_kernel_spmd

    h = np.ascontiguousarray(np.asarray(h, dtype=np.float32))
    W = np.asarray(W_self, dtype=np.float32)
    src = np.asarray(src, dtype=np.int64)
    dst = np.asarray(dst, dtype=np.int64)

    order = np.argsort(dst, kind="stable")
    src_s = src[order]
    dst_s = dst[order]

    # partition-interleaved Z row index of each edge's src node
    zrow_s = (src_s % 128) * TPB + (src_s // 128) % TPB
    zblk_s = src_s // NBN

    # per-(core, tile) edge ranges; tiles are 128 consecutive owned nodes
    tile_base = []
    for c in range(CORES):
        for t in range(NT):
            tile_base.append(c * NPC + t * 128)
    bounds_lo = np.searchsorted(dst_s, np.array(tile_base), side="left")
    hi_nodes = [min(b + 128, (b // NPC + 1) * NPC) for b in tile_base]
    bounds_hi = np.searchsorted(dst_s, np.array(hi_nodes), side="left")

    # split each tile's edges by src block; caps shared across cores (SPMD)
    per_ct = {}
    cnt = np.zeros((CORES, NT, NPB), dtype=np.int64)
    for c in range(CORES):
        for t in range(NT):
            i = c * NT + t
            lo, hi = int(bounds_lo[i]), int(bounds_hi[i])
            blk = zblk_s[lo:hi]
            for s_ in range(NPB):
                sel = np.nonzero(blk == s_)[0]
                zr = zrow_s[lo:hi][sel]
                dl = dst_s[lo:hi][sel] - tile_base[i]
                o2 = np.argsort(zr, kind="stable")   # ascending gather addrs
                per_ct[(c, t, s_)] = (zr[o2], dl[o2])
                cnt[c, t, s_] = len(sel)
    caps = [
        [int((cnt[:, t, s_].max() + 127) // 128) for s_ in range(NPB)]
        for t in range(NT)
    ]
    assert max(a + b for a, b in caps) <= 40, f"edge distribution too skewed: {caps}"
    NCH = sum(sum(r) for r in caps)

    # host-side layout prep
    import ml_dtypes
    bf = ml_dtypes.bfloat16
    h_pad = np.zeros((NPAD, D), dtype=np.float32)
    h_pad[:N_NODES] = h
    hT = np.ascontiguousarray(
        h_pad.astype(bf).T.reshape(2, 128, NT_ALL // SLAB, SLAB, 128)
        .transpose(1, 2, 3, 0, 4)
    )
    WIfull = np.zeros((D, 2 * D), dtype=np.float32)
    WIfull[:, :D] = W.T
    WIfull[np.arange(D), D + np.arange(D)] = 1.0
    WI = np.ascontiguousarray(
        WIfull.reshape(2, 128, 2 * D).transpose(1, 0, 2).astype(bf)
    )
    iota = np.broadcast_to(
        np.arange(128, dtype=np.float32)[None, :], (128, 128)
    ).copy()

    in_maps = []
    for c in range(CORES):
        idx_parts = []
        dstl = np.full((128, NCH), -1.0, dtype=np.float32)
        coff = 0
        for t in range(NT):
            for s_ in range(NPB):
                Cs = caps[t][s_]
                if Cs == 0:
                    continue
                CAPs = 128 * Cs
                zr, dl_real = per_ct[(c, t, s_)]
                n = len(zr)
                zpad = np.zeros(CAPs, dtype=np.int64)
                zpad[:n] = zr
                idx_parts.append(_wrap_idx(zpad))
                ei = np.arange(n)
                dstl[ei % 128, coff + ei // 128] = dl_real
                coff += Cs
        hown = np.zeros((NROWS, D), dtype=np.float32)
        hown[:NPC] = h[c * NPC:(c + 1) * NPC]
        in_maps.append({
            "hT": hT,
            "WI": WI,
            "iota": iota,
            "dstl": dstl,
            "idx": np.ascontiguousarray(np.concatenate(idx_parts, axis=1)),
            "hown": hown,
        })

    key = tuple(tuple(r) for r in caps)
    if key not in _cache:
        _cache[key] = _build(caps)
    nc = _cache[key]

    res = run_bass_kernel_spmd(nc, in_maps, core_ids=list(range(CORES)))
    out = np.concatenate(
        [res.results[c]["out"][:NPC] for c in range(CORES)], axis=0
    )
    return out.astype(np.float32)


# revision 11
# speedup vs baseline: 1.5854x; 1.0111x over previous
"""DeepSATConv GNN message-passing kernel for 8 Trainium2 NeuronCores.

Math note: the reference computes a per-channel segment-softmax over
msg = self_h[src] + neib_h[dst].  Within a dst-segment, neib_h[dst] (and
b_self, b_nb) are constant per channel, so they cancel in the softmax.
Hence alpha = segsoftmax(h @ W_self.T)[src] exactly, and
out[n] = segsum(e * h[src]) / segsum(e)  with e = exp((h @ W_self.T)[src]),
falling back to h[n] for zero-in-degree nodes.  W_nb / b_nb / b_self do
not affect the output at all.

Because e and e*h are pure per-NODE quantities, phase A precomputes
Z = [E | Y] = [exp(sh) | exp(sh)*h] in fp16 for all nodes (per-node work,
2.2x less than per-edge).  Phase B is then just a 1KB-per-edge dma_gather
of Z[src] plus one-hot selector matmuls in fp16 (1 PE cycle/row vs 4 for
fp32) accumulating [denom | numer] per 128-node tile in PSUM:
  acc = sum_chunks S_j.T @ Zx_j,  S_j[e, n] = (dst_local[e] == n)
Z lives in DRAM in a partition-interleaved layout (node n -> row
(n%128)*80 + (n//128)%80 of its half) so phase A can write 4 node-tiles
per DMA with 4KB contiguous per partition.

Sharding: nodes are permuted and bin-packed across the 8 cores x 20
tiles so every (core, tile, src-block) bin has a near-equal edge count
(SPMD caps are a max over cores, so balance directly cuts the padded
slot count and with it Q7 descriptor-generation time, the kernel's
critical resource).  Edges are partitioned by destination node so
segment reductions stay core-local; phase A is replicated (cheaper than
collectives at this size).
"""

import numpy as np

N_NODES = 20000
N_EDGES = 320000
D = 256
CORES = 8
NPC = N_NODES // CORES          # 2500 nodes per core
NT = (NPC + 127) // 128         # 20 node tiles per core
NROWS = NT * 128                # 2560 padded rows per core
NT_ALL = 160                    # phase-A tiles over all nodes
NPAD = NT_ALL * 128             # 20480
NPB = 2                         # Z source blocks (phase A/B overlap)
TPB = NT_ALL // NPB             # 80 tiles per Z block
NBN = 128 * TPB                 # 10240 nodes per Z block
SLAB = 8                        # phase-A hT tiles per DMA load
WB = 4                          # Z tiles buffered per DMA write

_cache = {}


def _build(caps):
    import concourse.bacc as bacc
    import concourse.mybir as mybir
    from concourse.tile import TileContext

    nc = bacc.Bacc("TRN2")
    f32 = mybir.dt.float32
    f16 = mybir.dt.float16
    bf16 = mybir.dt.bfloat16

    NCH = sum(sum(r) for r in caps)     # total chunks across tiles/blocks
    NIX = 128 * NCH                     # total gathered edge slots
    hT_d = nc.dram_tensor(
        "hT", [128, NT_ALL // SLAB, SLAB, 2, 128], bf16, kind="ExternalInput"
    )
    WI_d = nc.dram_tensor("WI", [128, 2, 2 * D], bf16, kind="ExternalInput")
    S_d = nc.dram_tensor("S", [128, NCH, 128], f16, kind="ExternalInput")
    idx_d = nc.dram_tensor("idx", [128, NIX // 16], mybir.dt.int16, kind="ExternalInput")
    hown_d = nc.dram_tensor("hown", [NROWS, D], f32, kind="ExternalInput")
    out_d = nc.dram_tensor("out", [NROWS, D], f32, kind="ExternalOutput")

    CMAX = max(a + b for a, b in caps)
    with TileContext(nc) as tc:
        with (
            tc.tile_pool(name="const", bufs=1) as constp,
            tc.tile_pool(name="pha", bufs=3) as pha,
            tc.tile_pool(name="phz", bufs=3) as phz,
            tc.tile_pool(name="gat", bufs=3) as gat,
            tc.tile_pool(name="wrk", bufs=8) as wrk,
            tc.tile_pool(name="sgp", bufs=3) as sgp,
            tc.tile_pool(name="fin", bufs=2) as fin,
            tc.tile_pool(name="psa", bufs=3, space="PSUM") as psa,
            tc.tile_pool(name="psb", bufs=3, space="PSUM") as psb,
            tc.tile_pool(name="dram", bufs=1, space="DRAM") as dramp,
        ):
            z_blk = [
                dramp.tile(
                    [128, TPB, 2 * D], f16, tag=f"zblk{s_}", name=f"zblk{s_}"
                )
                for s_ in range(NPB)
            ]

            # ---- constants ----
            WI_sb = constp.tile([128, 2, 2 * D], bf16)
            nc.sync.dma_start(WI_sb[:, :, :], WI_d[:, :, :])
            idx_sb = constp.tile([128, NIX // 16], mybir.dt.int16)
            nc.sync.dma_start(idx_sb[:, :], idx_d[:, :])

            # ---- phase A: Z = [exp(sh) | exp(sh)*h], sh = h @ W_self.T ----
            # one bf16 product (~0.4% on exp, averaged down by the softmax);
            # h reaches PSUM fp32 via the [W.T | I] identity columns.
            zq = None
            for g in range(NT_ALL // SLAB):
                hT_sb = pha.tile([128, SLAB, 2, 128], bf16, tag="hT")
                nc.sync.dma_start(hT_sb[:, :, :, :], hT_d[:, g, :, :, :])
                for j in range(SLAB):
                    i = g * SLAB + j
                    ps = psa.tile([128, 2 * D], f32, tag="ps")
                    for kb in range(2):
                        nc.tensor.matmul(
                            ps[:, :], hT_sb[:, j, kb, :], WI_sb[:, kb, :],
                            start=(kb == 0), stop=(kb == 1),
                        )
                    if i % WB == 0:
                        zq = phz.tile([128, WB, 2 * D], f16, tag="zq")
                    w = i % WB
                    e32 = wrk.tile([128, D], f32, tag="e32")
                    nc.scalar.activation(
                        e32[:, :], ps[:, 0:D], mybir.ActivationFunctionType.Exp
                    )
                    nc.vector.tensor_tensor(
                        zq[:, w, D:2 * D], e32[:, :], ps[:, D:2 * D],
                        mybir.AluOpType.mult,
                    )
                    nc.scalar.copy(zq[:, w, 0:D], e32[:, :])
                    if w == WB - 1:
                        s_, tb = divmod(i, TPB)
                        nc.sync.dma_start(
                            z_blk[s_][:, tb - (WB - 1):tb + 1, :], zq[:, :, :]
                        )

            # ---- phase B: per node-tile gather + segment softmax-sum ----
            chunk_off = 0   # global chunk counter (indexes idx/S layout)
            for t in range(NT):
                zx_t = gat.tile([128, CMAX, 2 * D], f16, tag="zx")
                C_t = caps[t][0] + caps[t][1]
                zoff = 0
                for s_ in range(NPB):
                    Cs = caps[t][s_]
                    if Cs == 0:
                        continue
                    CAPs = 128 * Cs
                    io = (chunk_off + zoff) * 8
                    nc.gpsimd.dma_gather(
                        zx_t[:, zoff:zoff + Cs, :],
                        z_blk[s_][:, :, :].flatten_outer_dims(),
                        idx_sb[:, io:io + 8 * Cs], CAPs, CAPs, 2 * D,
                        single_packet=False,
                    )
                    zoff += Cs
                Sg = sgp.tile([128, CMAX, 128], f16, tag="Sg")
                nc.sync.dma_start(
                    Sg[:, 0:C_t, :], S_d[:, chunk_off:chunk_off + C_t, :]
                )
                acc = psb.tile([128, 2 * D], f32, tag="acc")
                for j in range(C_t):
                    nc.tensor.matmul(
                        acc[:, :], Sg[:, j, :], zx_t[:, j, :],
                        start=(j == 0), stop=(j == C_t - 1),
                    )
                chunk_off += C_t

                # ---- finalize tile: out = numer/denom, h for empty nodes ----
                dmax = fin.tile([128, D], f32, tag="dmax")
                nc.vector.tensor_scalar(
                    dmax[:, :], acc[:, 0:D], 1e-37, None, mybir.AluOpType.max
                )
                rec = fin.tile([128, D], f32, tag="rec")
                nc.vector.reciprocal(rec[:, :], dmax[:, :])
                res = fin.tile([128, D], f32, tag="res")
                nc.vector.tensor_tensor(
                    res[:, :], acc[:, D:2 * D], rec[:, :], mybir.AluOpType.mult
                )
                mask = fin.tile([128, D], mybir.dt.uint8, tag="mask")
                nc.vector.tensor_scalar(
                    mask[:, :], acc[:, 0:D], 0.0, None, mybir.AluOpType.is_equal
                )
                hown_sb = fin.tile([128, D], f32, tag="hown")
                nc.sync.dma_start(hown_sb[:, :], hown_d[t * 128:(t + 1) * 128, :])
                nc.vector.copy_predicated(res[:, :], mask[:, :], hown_sb[:, :])
                nc.sync.dma_start(out_d[t * 128:(t + 1) * 128, :], res[:, :])
    nc.compile()
    return nc


def _wrap_idx(ix):
    # dma_gather index layout: logical index i lands at output
    # [partition i%128, slot i//128]; the SBUF index tile stores it at
    # [i%16, 8*(i//128) + (i%128)//16], replicated over the 8 Q7 cores.
    w = ix.astype(np.int16).reshape(-1, 8, 16).transpose(2, 0, 1).reshape(16, -1)
    return np.tile(w, (8, 1))


def _balance_nodes(deg_blk, limit=1024):
    """Assign nodes to CORES*NT bins (<=128 nodes each, occupancy free) so
    per-(bin, src-block) edge counts stay <= limit.  deg_blk: [N_NODES, NPB]
    in-degree split by src block.  Returns assign[node] = bin."""
    nbins = CORES * NT
    order = np.argsort(-deg_blk.sum(axis=1), kind="stable")
    fill = np.zeros((nbins, NPB), dtype=np.int64)
    count = np.zeros(nbins, dtype=np.int64)
    assign = np.empty(N_NODES, dtype=np.int64)
    pos = 0
    while pos < N_NODES:
        # wave greedy: next wave of heavy nodes onto the least-loaded bins
        wave = order[pos:pos + nbins]
        avail = np.nonzero(count < 128)[0]
        ranked = avail[np.argsort(
            fill[avail].max(axis=1) * 128 + count[avail], kind="stable")]
        k = min(len(wave), len(ranked))
        assign[wave[:k]] = ranked[:k]
        np.add.at(fill, (ranked[:k],), deg_blk[wave[:k]])
        np.add.at(count, ranked[:k], 1)
        pos += k
    # move repair: shift single nodes out of overfull (bin, block) cells
    for _ in range(4000):
        b = int(np.argmax(fill.max(axis=1)))
        if fill[b].max() <= limit:
            break
        s = int(np.argmax(fill[b]))
        over = fill[b, s] - limit
        nodes_b = np.nonzero(assign == b)[0]
        db = deg_blk[nodes_b]
        room = (count < 128) & (np.arange(nbins) != b)
        tgt = np.nonzero(room)[0]
        if len(tgt) == 0:
            break
        # smallest node that clears the overshoot in one move (else biggest)
        ds = db[:, s]
        clr = np.nonzero(ds >= over)[0]
        ni = (clr[np.argmin(db[clr].sum(axis=1))] if len(clr)
              else int(np.argmax(ds)))
        n = nodes_b[ni]
        ok = tgt[((fill[tgt] + deg_blk[n]).max(axis=1) <= limit)]
        if len(ok) == 0:
            break
        dest = ok[np.argmin(fill[ok].max(axis=1) * 256 + count[ok])]
        fill[b] -= deg_blk[n]
        fill[dest] += deg_blk[n]
        count[b] -= 1
        count[dest] += 1
        assign[n] = dest
    return assign


def kernel(h, W_nb, b_nb, W_self, b_self, src, dst):
    from concourse.bass_utils import run_bass_kernel_spmd

    h = np.ascontiguousarray(np.asarray(h, dtype=np.float32))
    W = np.asarray(W_self, dtype=np.float32)
    src = np.asarray(src, dtype=np.int64)
    dst = np.asarray(dst, dtype=np.int64)

    # partition-interleaved Z row of each node (original id space; phase A
    # and the gather side are unaffected by the dst rebalancing permutation)
    zrow_e = (src % 128) * TPB + (src // 128) % TPB
    zblk_e = src // NBN

    # --- rebalance dst nodes across (core, tile) bins ---
    # node -> row (bin*128 + slot) with free per-bin occupancy; empty rows
    # get denom 0 on device and are dropped by the host unshard.
    deg_blk = np.zeros((N_NODES, NPB), dtype=np.int64)
    np.add.at(deg_blk, (dst, zblk_e), 1)
    assign = _balance_nodes(deg_blk)          # node -> bin
    o_bin = np.argsort(assign, kind="stable")
    slot = np.arange(N_NODES) - np.searchsorted(
        assign[o_bin], assign[o_bin], side="left"
    )
    noderow = np.empty(N_NODES, dtype=np.int64)
    noderow[o_bin] = assign[o_bin] * 128 + slot          # node -> row
    rownode = np.full(CORES * NROWS, -1, dtype=np.int64)
    rownode[noderow] = np.arange(N_NODES)                # row -> node | -1
    dstb = noderow[dst]                                  # balanced dst rows

    order = np.argsort(dstb, kind="stable")
    src_s = src[order]
    dstb_s = dstb[order]
    zrow_s = zrow_e[order]
    zblk_s = zblk_e[order]

    # per-(core, tile) edge ranges; tiles are 128 consecutive balanced rows
    tile_base = np.arange(CORES * NT) * 128
    bounds_lo = np.searchsorted(dstb_s, tile_base, side="left")
    bounds_hi = np.searchsorted(dstb_s, tile_base + 128, side="left")

    # split each tile's edges by src block; caps shared across cores (SPMD)
    per_ct = {}
    cnt = np.zeros((CORES, NT, NPB), dtype=np.int64)
    for c in range(CORES):
        for t in range(NT):
            i = c * NT + t
            lo, hi = int(bounds_lo[i]), int(bounds_hi[i])
            blk = zblk_s[lo:hi]
            for s_ in range(NPB):
                sel = np.nonzero(blk == s_)[0]
                zr = zrow_s[lo:hi][sel]
                dl = dstb_s[lo:hi][sel] - tile_base[i]
                o2 = np.argsort(zr, kind="stable")   # ascending gather addrs
                per_ct[(c, t, s_)] = (zr[o2], dl[o2])
                cnt[c, t, s_] = len(sel)
    caps = [
        [int((cnt[:, t, s_].max() + 127) // 128) for s_ in range(NPB)]
        for t in range(NT)
    ]
    assert max(a + b for a, b in caps) <= 40, f"edge distribution too skewed: {caps}"
    NCH = sum(sum(r) for r in caps)

    # host-side layout prep
    import ml_dtypes
    bf = ml_dtypes.bfloat16
    h_pad = np.zeros((NPAD, D), dtype=np.float32)
    h_pad[:N_NODES] = h
    hT = np.ascontiguousarray(
        h_pad.astype(bf).T.reshape(2, 128, NT_ALL // SLAB, SLAB, 128)
        .transpose(1, 2, 3, 0, 4)
    )
    WIfull = np.zeros((D, 2 * D), dtype=np.float32)
    WIfull[:, :D] = W.T
    WIfull[np.arange(D), D + np.arange(D)] = 1.0
    WI = np.ascontiguousarray(
        WIfull.reshape(2, 128, 2 * D).transpose(1, 0, 2).astype(bf)
    )

    in_maps = []
    for c in range(CORES):
        idx_parts = []
        S_all = np.zeros((128, NCH, 128), dtype=np.float16)
        coff = 0
        for t in range(NT):
            for s_ in range(NPB):
                Cs = caps[t][s_]
                if Cs == 0:
                    continue
                CAPs = 128 * Cs
                zr, dl = per_ct[(c, t, s_)]
                n = len(zr)
                zpad = np.zeros(CAPs, dtype=np.int64)
                zpad[:n] = zr
                idx_parts.append(_wrap_idx(zpad))
                ei = np.arange(n)
                S_all[ei % 128, coff + ei // 128, dl] = 1.0
                coff += Cs
        rn = rownode[c * NROWS:(c + 1) * NROWS]
        hown = np.zeros((NROWS, D), dtype=np.float32)
        hown[rn >= 0] = h[rn[rn >= 0]]
        in_maps.append({
            "hT": hT,
            "WI": WI,
            "S": S_all,
            "idx": np.ascontiguousarray(np.concatenate(idx_parts, axis=1)),
            "hown": hown,
        })

    key = tuple(tuple(r) for r in caps)
    if key not in _cache:
        _cache[key] = _build(caps)
    nc = _cache[key]

    res = run_bass_kernel_spmd(nc, in_maps, core_ids=list(range(CORES)))
    outb = np.concatenate(
        [res.results[c]["out"] for c in range(CORES)], axis=0
    )
    out = np.empty((N_NODES, D), dtype=np.float32)
    valid = rownode >= 0
    out[rownode[valid]] = outb[valid]         # un-permute balanced rows
    return out.astype(np.float32)


# revision 14
# speedup vs baseline: 1.7560x; 1.1076x over previous
"""DeepSATConv GNN message-passing kernel for 8 Trainium2 NeuronCores.

Math note: the reference computes a per-channel segment-softmax over
msg = self_h[src] + neib_h[dst].  Within a dst-segment, neib_h[dst] (and
b_self, b_nb) are constant per channel, so they cancel in the softmax.
Hence alpha = segsoftmax(h @ W_self.T)[src] exactly, and
out[n] = segsum(e * h[src]) / segsum(e)  with e = exp((h @ W_self.T)[src]),
falling back to h[n] for zero-in-degree nodes.  W_nb / b_nb / b_self do
not affect the output at all.

Because e and e*h are pure per-NODE quantities, phase A precomputes
Z = [E | Y] = [exp(sh) | exp(sh)*h] in fp16 for all nodes (per-node work,
2.2x less than per-edge).  Phase B is then just a 1KB-per-edge dma_gather
of Z[src] plus one-hot selector matmuls in fp16 (1 PE cycle/row vs 4 for
fp32) accumulating [denom | numer] per 128-node tile in PSUM:
  acc = sum_chunks S_j.T @ Zx_j,  S_j[e, n] = (dst_local[e] == n)
Z lives in DRAM in a partition-interleaved layout (node n -> row
(n%128)*80 + (n//128)%80 of its half) so phase A can write 4 node-tiles
per DMA with 4KB contiguous per partition.

Sharding: nodes are permuted and bin-packed across the 8 cores x 20
tiles so every (core, tile, src-block) bin has a near-equal edge count
(SPMD caps are a max over cores, so balance directly cuts the padded
slot count and with it Q7 descriptor-generation time, the kernel's
critical resource).  Edges are partitioned by destination node so
segment reductions stay core-local; phase A is replicated (cheaper than
collectives at this size).
"""

import numpy as np

N_NODES = 20000
N_EDGES = 320000
D = 256
CORES = 8
NPC = N_NODES // CORES          # 2500 nodes per core
NT = (NPC + 127) // 128         # 20 node tiles per core
NROWS = NT * 128                # 2560 padded rows per core
NT_ALL = 160                    # phase-A tiles over all nodes
NPAD = NT_ALL * 128             # 20480
NPB = 2                         # Z source blocks (phase A/B overlap)
TPB = NT_ALL // NPB             # 80 tiles per Z block
NBN = 128 * TPB                 # 10240 nodes per Z block
SLAB = 8                        # phase-A hT tiles per DMA load
WB = 4                          # Z tiles buffered per DMA write
DELTA = 8                       # blk0 gathers issued ahead of blk1

_cache = {}


def _build(caps):
    import concourse.bacc as bacc
    import concourse.mybir as mybir
    from concourse.tile import TileContext

    nc = bacc.Bacc("TRN2")
    f32 = mybir.dt.float32
    f16 = mybir.dt.float16
    bf16 = mybir.dt.bfloat16

    NCH = sum(sum(r) for r in caps)     # total chunks across tiles/blocks
    NIX = 128 * NCH                     # total gathered edge slots
    hT_d = nc.dram_tensor(
        "hT", [128, NT_ALL // SLAB, SLAB, 2, 128], bf16, kind="ExternalInput"
    )
    WI_d = nc.dram_tensor("WI", [128, 2, 2 * D], bf16, kind="ExternalInput")
    S_d = nc.dram_tensor("S", [128, NCH, 128], f16, kind="ExternalInput")
    idx_d = nc.dram_tensor("idx", [128, NIX // 16], mybir.dt.int16, kind="ExternalInput")
    hown_d = nc.dram_tensor("hown", [NROWS, D], f32, kind="ExternalInput")
    out_d = nc.dram_tensor("out", [NROWS, D], f32, kind="ExternalOutput")

    CMAX = max(a + b for a, b in caps)
    with TileContext(nc) as tc:
        with (
            tc.tile_pool(name="const", bufs=1) as constp,
            tc.tile_pool(name="pha", bufs=3) as pha,
            tc.tile_pool(name="phz", bufs=3) as phz,
            tc.tile_pool(name="gat", bufs=DELTA + 2) as gat,
            tc.tile_pool(name="gat1", bufs=2) as gat1,
            tc.tile_pool(name="wrk", bufs=8) as wrk,
            tc.tile_pool(name="sgp", bufs=3) as sgp,
            tc.tile_pool(name="fin", bufs=2) as fin,
            tc.tile_pool(name="psa", bufs=3, space="PSUM") as psa,
            tc.tile_pool(name="psb", bufs=3, space="PSUM") as psb,
            tc.tile_pool(name="dram", bufs=1, space="DRAM") as dramp,
        ):
            z_blk = [
                dramp.tile(
                    [128, TPB, 2 * D], f16, tag=f"zblk{s_}", name=f"zblk{s_}"
                )
                for s_ in range(NPB)
            ]

            # ---- constants ----
            WI_sb = constp.tile([128, 2, 2 * D], bf16)
            nc.sync.dma_start(WI_sb[:, :, :], WI_d[:, :, :])
            idx_sb = constp.tile([128, NIX // 16], mybir.dt.int16)
            nc.sync.dma_start(idx_sb[:, :], idx_d[:, :])

            # ---- phase A: Z = [exp(sh) | exp(sh)*h], sh = h @ W_self.T ----
            # one bf16 product (~0.4% on exp, averaged down by the softmax);
            # h reaches PSUM fp32 via the [W.T | I] identity columns.
            zq = None
            for g in range(NT_ALL // SLAB):
                hT_sb = pha.tile([128, SLAB, 2, 128], bf16, tag="hT")
                nc.sync.dma_start(hT_sb[:, :, :, :], hT_d[:, g, :, :, :])
                for j in range(SLAB):
                    i = g * SLAB + j
                    ps = psa.tile([128, 2 * D], f32, tag="ps")
                    for kb in range(2):
                        nc.tensor.matmul(
                            ps[:, :], hT_sb[:, j, kb, :], WI_sb[:, kb, :],
                            start=(kb == 0), stop=(kb == 1),
                        )
                    if i % WB == 0:
                        zq = phz.tile([128, WB, 2 * D], f16, tag="zq")
                    w = i % WB
                    e32 = wrk.tile([128, D], f32, tag="e32")
                    nc.scalar.activation(
                        e32[:, :], ps[:, 0:D], mybir.ActivationFunctionType.Exp
                    )
                    nc.vector.tensor_tensor(
                        zq[:, w, D:2 * D], e32[:, :], ps[:, D:2 * D],
                        mybir.AluOpType.mult,
                    )
                    nc.scalar.copy(zq[:, w, 0:D], e32[:, :])
                    if w == WB - 1:
                        s_, tb = divmod(i, TPB)
                        nc.sync.dma_start(
                            z_blk[s_][:, tb - (WB - 1):tb + 1, :], zq[:, :, :]
                        )

            # ---- phase B: per node-tile gather + segment softmax-sum ----
            # blk1 is only ready once phase A finishes; issue DELTA tiles of
            # blk0 gathers ahead so the in-order gpsimd queue stays busy
            # instead of stalling behind the first blk1 gather.
            offs = []
            o = 0
            for t in range(NT):
                offs.append([o, o + caps[t][0]])
                o += caps[t][0] + caps[t][1]
            C0M = max(c for c, _ in caps)
            C1M = max(c for _, c in caps)

            zx0_t = {}

            def issue_gather(t, s_, dest):
                Cs = caps[t][s_]
                if Cs == 0:
                    return
                CAPs = 128 * Cs
                io = offs[t][s_] * 8
                nc.gpsimd.dma_gather(
                    dest[:, 0:Cs, :],
                    z_blk[s_][:, :, :].flatten_outer_dims(),
                    idx_sb[:, io:io + 8 * Cs], CAPs, CAPs, 2 * D,
                    single_packet=False,
                )

            for t in range(DELTA):
                zx0_t[t] = gat.tile([128, C0M, 2 * D], f16, tag="zx0", name="zx0")
                issue_gather(t, 0, zx0_t[t])
            for t in range(NT):
                if t + DELTA < NT:
                    zx0_t[t + DELTA] = gat.tile([128, C0M, 2 * D], f16, tag="zx0", name="zx0")
                    issue_gather(t + DELTA, 0, zx0_t[t + DELTA])
                zx1 = gat1.tile([128, C1M, 2 * D], f16, tag="zx1")
                issue_gather(t, 1, zx1)
                c0 = caps[t][0]
                C_t = caps[t][0] + caps[t][1]
                Sg = sgp.tile([128, CMAX, 128], f16, tag="Sg")
                nc.sync.dma_start(
                    Sg[:, 0:C_t, :], S_d[:, offs[t][0]:offs[t][0] + C_t, :]
                )
                acc = psb.tile([128, 2 * D], f32, tag="acc")
                for j in range(C_t):
                    rhs = (zx0_t[t][:, j, :] if j < c0
                           else zx1[:, j - c0, :])
                    nc.tensor.matmul(
                        acc[:, :], Sg[:, j, :], rhs,
                        start=(j == 0), stop=(j == C_t - 1),
                    )
                zx0_t.pop(t)

                # ---- finalize tile: out = numer/denom, h for empty nodes ----
                dmax = fin.tile([128, D], f32, tag="dmax")
                nc.vector.tensor_scalar(
                    dmax[:, :], acc[:, 0:D], 1e-37, None, mybir.AluOpType.max
                )
                rec = fin.tile([128, D], f32, tag="rec")
                nc.vector.reciprocal(rec[:, :], dmax[:, :])
                res = fin.tile([128, D], f32, tag="res")
                nc.vector.tensor_tensor(
                    res[:, :], acc[:, D:2 * D], rec[:, :], mybir.AluOpType.mult
                )
                mask = fin.tile([128, D], mybir.dt.uint8, tag="mask")
                nc.vector.tensor_scalar(
                    mask[:, :], acc[:, 0:D], 0.0, None, mybir.AluOpType.is_equal
                )
                hown_sb = fin.tile([128, D], f32, tag="hown")
                nc.sync.dma_start(hown_sb[:, :], hown_d[t * 128:(t + 1) * 128, :])
                nc.vector.copy_predicated(res[:, :], mask[:, :], hown_sb[:, :])
                nc.sync.dma_start(out_d[t * 128:(t + 1) * 128, :], res[:, :])
    nc.compile()
    return nc


def _wrap_idx(ix):
    # dma_gather index layout: logical index i lands at output
    # [partition i%128, slot i//128]; the SBUF index tile stores it at
    # [i%16, 8*(i//128) + (i%128)//16], replicated over the 8 Q7 cores.
    w = ix.astype(np.int16).reshape(-1, 8, 16).transpose(2, 0, 1).reshape(16, -1)
    return np.tile(w, (8, 1))


def _balance_nodes(deg_blk, limit=1024):
    """Assign nodes to CORES*NT bins (<=128 nodes each, occupancy free) so
    per-(bin, src-block) edge counts stay <= limit.  deg_blk: [N_NODES, NPB]
    in-degree split by src block.  Returns assign[node] = bin."""
    nbins = CORES * NT
    order = np.argsort(-deg_blk.sum(axis=1), kind="stable")
    fill = np.zeros((nbins, NPB), dtype=np.int64)
    count = np.zeros(nbins, dtype=np.int64)
    assign = np.empty(N_NODES, dtype=np.int64)
    pos = 0
    while pos < N_NODES:
        # wave greedy: next wave of heavy nodes onto the least-loaded bins
        wave = order[pos:pos + nbins]
        avail = np.nonzero(count < 128)[0]
        ranked = avail[np.argsort(
            fill[avail].max(axis=1) * 128 + count[avail], kind="stable")]
        k = min(len(wave), len(ranked))
        assign[wave[:k]] = ranked[:k]
        np.add.at(fill, (ranked[:k],), deg_blk[wave[:k]])
        np.add.at(count, ranked[:k], 1)
        pos += k
    # move repair: shift single nodes out of overfull (bin, block) cells
    for _ in range(4000):
        b = int(np.argmax(fill.max(axis=1)))
        if fill[b].max() <= limit:
            break
        s = int(np.argmax(fill[b]))
        over = fill[b, s] - limit
        nodes_b = np.nonzero(assign == b)[0]
        db = deg_blk[nodes_b]
        room = (count < 128) & (np.arange(nbins) != b)
        tgt = np.nonzero(room)[0]
        if len(tgt) == 0:
            break
        # smallest node that clears the overshoot in one move (else biggest)
        ds = db[:, s]
        clr = np.nonzero(ds >= over)[0]
        ni = (clr[np.argmin(db[clr].sum(axis=1))] if len(clr)
              else int(np.argmax(ds)))
        n = nodes_b[ni]
        ok = tgt[((fill[tgt] + deg_blk[n]).max(axis=1) <= limit)]
        if len(ok) == 0:
            break
        dest = ok[np.argmin(fill[ok].max(axis=1) * 256 + count[ok])]
        fill[b] -= deg_blk[n]
        fill[dest] += deg_blk[n]
        count[b] -= 1
        count[dest] += 1
        assign[n] = dest
    return assign


def kernel(h, W_nb, b_nb, W_self, b_self, src, dst):
    from concourse.bass_utils import run_bass_kernel_spmd

    h = np.ascontiguousarray(np.asarray(h, dtype=np.float32))
    W = np.asarray(W_self, dtype=np.float32)
    src = np.asarray(src, dtype=np.int64)
    dst = np.asarray(dst, dtype=np.int64)

    # partition-interleaved Z row of each node (original id space; phase A
    # and the gather side are unaffected by the dst rebalancing permutation)
    zrow_e = (src % 128) * TPB + (src // 128) % TPB
    zblk_e = src // NBN

    # --- rebalance dst nodes across (core, tile) bins ---
    # node -> row (bin*128 + slot) with free per-bin occupancy; empty rows
    # get denom 0 on device and are dropped by the host unshard.
    deg_blk = np.zeros((N_NODES, NPB), dtype=np.int64)
    np.add.at(deg_blk, (dst, zblk_e), 1)
    assign = _balance_nodes(deg_blk)          # node -> bin
    o_bin = np.argsort(assign, kind="stable")
    slot = np.arange(N_NODES) - np.searchsorted(
        assign[o_bin], assign[o_bin], side="left"
    )
    noderow = np.empty(N_NODES, dtype=np.int64)
    noderow[o_bin] = assign[o_bin] * 128 + slot          # node -> row
    rownode = np.full(CORES * NROWS, -1, dtype=np.int64)
    rownode[noderow] = np.arange(N_NODES)                # row -> node | -1
    dstb = noderow[dst]                                  # balanced dst rows

    order = np.argsort(dstb, kind="stable")
    src_s = src[order]
    dstb_s = dstb[order]
    zrow_s = zrow_e[order]
    zblk_s = zblk_e[order]

    # per-(core, tile) edge ranges; tiles are 128 consecutive balanced rows
    tile_base = np.arange(CORES * NT) * 128
    bounds_lo = np.searchsorted(dstb_s, tile_base, side="left")
    bounds_hi = np.searchsorted(dstb_s, tile_base + 128, side="left")

    # split each tile's edges by src block; caps shared across cores (SPMD)
    per_ct = {}
    cnt = np.zeros((CORES, NT, NPB), dtype=np.int64)
    for c in range(CORES):
        for t in range(NT):
            i = c * NT + t
            lo, hi = int(bounds_lo[i]), int(bounds_hi[i])
            blk = zblk_s[lo:hi]
            for s_ in range(NPB):
                sel = np.nonzero(blk == s_)[0]
                zr = zrow_s[lo:hi][sel]
                dl = dstb_s[lo:hi][sel] - tile_base[i]
                o2 = np.argsort(zr, kind="stable")   # ascending gather addrs
                per_ct[(c, t, s_)] = (zr[o2], dl[o2])
                cnt[c, t, s_] = len(sel)
    caps = [
        [int((cnt[:, t, s_].max() + 127) // 128) for s_ in range(NPB)]
        for t in range(NT)
    ]
    assert max(a + b for a, b in caps) <= 40, f"edge distribution too skewed: {caps}"
    NCH = sum(sum(r) for r in caps)

    # host-side layout prep
    import ml_dtypes
    bf = ml_dtypes.bfloat16
    h_pad = np.zeros((NPAD, D), dtype=np.float32)
    h_pad[:N_NODES] = h
    hT = np.ascontiguousarray(
        h_pad.astype(bf).T.reshape(2, 128, NT_ALL // SLAB, SLAB, 128)
        .transpose(1, 2, 3, 0, 4)
    )
    WIfull = np.zeros((D, 2 * D), dtype=np.float32)
    WIfull[:, :D] = W.T
    WIfull[np.arange(D), D + np.arange(D)] = 1.0
    WI = np.ascontiguousarray(
        WIfull.reshape(2, 128, 2 * D).transpose(1, 0, 2).astype(bf)
    )

    in_maps = []
    for c in range(CORES):
        idx_parts = []
        S_all = np.zeros((128, NCH, 128), dtype=np.float16)
        coff = 0
        for t in range(NT):
            for s_ in range(NPB):
                Cs = caps[t][s_]
                if Cs == 0:
                    continue
                CAPs = 128 * Cs
                zr, dl = per_ct[(c, t, s_)]
                n = len(zr)
                zpad = np.zeros(CAPs, dtype=np.int64)
                zpad[:n] = zr
                idx_parts.append(_wrap_idx(zpad))
                ei = np.arange(n)
                S_all[ei % 128, coff + ei // 128, dl] = 1.0
                coff += Cs
        rn = rownode[c * NROWS:(c + 1) * NROWS]
        hown = np.zeros((NROWS, D), dtype=np.float32)
        hown[rn >= 0] = h[rn[rn >= 0]]
        in_maps.append({
            "hT": hT,
            "WI": WI,
            "S": S_all,
            "idx": np.ascontiguousarray(np.concatenate(idx_parts, axis=1)),
            "hown": hown,
        })

    key = tuple(tuple(r) for r in caps)
    if key not in _cache:
        _cache[key] = _build(caps)
    nc = _cache[key]

    res = run_bass_kernel_spmd(nc, in_maps, core_ids=list(range(CORES)))
    outb = np.concatenate(
        [res.results[c]["out"] for c in range(CORES)], axis=0
    )
    out = np.empty((N_NODES, D), dtype=np.float32)
    valid = rownode >= 0
    out[rownode[valid]] = outb[valid]         # un-permute balanced rows
    return out.astype(np.float32)


# revision 16
# speedup vs baseline: 1.8111x; 1.0314x over previous
"""DeepSATConv GNN message-passing kernel for 8 Trainium2 NeuronCores.

Math note: the reference computes a per-channel segment-softmax over
msg = self_h[src] + neib_h[dst].  Within a dst-segment, neib_h[dst] (and
b_self, b_nb) are constant per channel, so they cancel in the softmax.
Hence alpha = segsoftmax(h @ W_self.T)[src] exactly, and
out[n] = segsum(e * h[src]) / segsum(e)  with e = exp((h @ W_self.T)[src]),
falling back to h[n] for zero-in-degree nodes.  W_nb / b_nb / b_self do
not affect the output at all.

Because e and e*h are pure per-NODE quantities, phase A precomputes
Z = [E | Y] = [exp(sh) | exp(sh)*h] in fp16 for all nodes (per-node work,
2.2x less than per-edge).  Phase B is then just a 1KB-per-edge dma_gather
of Z[src] plus one-hot selector matmuls in fp16 (1 PE cycle/row vs 4 for
fp32) accumulating [denom | numer] per 128-node tile in PSUM:
  acc = sum_chunks S_j.T @ Zx_j,  S_j[e, n] = (dst_local[e] == n)
Z lives in DRAM in a partition-interleaved layout (node n -> row
(n%128)*80 + (n//128)%80 of its half) so phase A can write 4 node-tiles
per DMA with 4KB contiguous per partition.

Sharding: nodes are permuted and bin-packed across the 8 cores x 20
tiles so every (core, tile, src-block) bin has a near-equal edge count
(SPMD caps are a max over cores, so balance directly cuts the padded
slot count and with it Q7 descriptor-generation time, the kernel's
critical resource).  Edges are partitioned by destination node so
segment reductions stay core-local; phase A is replicated (cheaper than
collectives at this size).
"""

import numpy as np

N_NODES = 20000
N_EDGES = 320000
D = 256
CORES = 8
NPC = N_NODES // CORES          # 2500 nodes per core
NT = (NPC + 127) // 128         # 20 node tiles per core
NROWS = NT * 128                # 2560 padded rows per core
NT_ALL = 160                    # phase-A tiles over all nodes
NPAD = NT_ALL * 128             # 20480
NPB = 2                         # Z source blocks (phase A/B overlap)
TB = (52, 108)                  # tiles per Z block (blk0 small: early gathers)
TS = (0, 52)                    # first tile of each block
NBN0 = 128 * TB[0]              # 6656 nodes in block 0
SLAB = 8                        # phase-A hT tiles per DMA load
WB = 4                          # Z tiles buffered per DMA write
DELTA = 16                      # blk0 gathers issued ahead of blk1
LIMITS = (6 * 128, 12 * 128)    # per-(bin, block) edge caps for the balancer

_cache = {}


def _build(caps):
    import concourse.bacc as bacc
    import concourse.mybir as mybir
    from concourse.tile import TileContext

    nc = bacc.Bacc("TRN2")
    f32 = mybir.dt.float32
    f16 = mybir.dt.float16
    bf16 = mybir.dt.bfloat16

    NCH = sum(sum(r) for r in caps)     # total chunks across tiles/blocks
    NIX = 128 * NCH                     # total gathered edge slots
    hT_d = nc.dram_tensor(
        "hT", [128, NT_ALL // SLAB, SLAB, 2, 128], bf16, kind="ExternalInput"
    )
    WI_d = nc.dram_tensor("WI", [128, 2, 2 * D], bf16, kind="ExternalInput")
    S_d = nc.dram_tensor("S", [128, NCH, 128], f16, kind="ExternalInput")
    idx_d = nc.dram_tensor("idx", [128, NIX // 16], mybir.dt.int16, kind="ExternalInput")
    hown_d = nc.dram_tensor("hown", [NROWS, D], f32, kind="ExternalInput")
    out_d = nc.dram_tensor("out", [NROWS, D], f32, kind="ExternalOutput")

    CMAX = max(a + b for a, b in caps)
    with TileContext(nc) as tc:
        with (
            tc.tile_pool(name="const", bufs=1) as constp,
            tc.tile_pool(name="pha", bufs=3) as pha,
            tc.tile_pool(name="phz", bufs=3) as phz,
            tc.tile_pool(name="gat", bufs=DELTA + 2) as gat,
            tc.tile_pool(name="gat1", bufs=2) as gat1,
            tc.tile_pool(name="wrk", bufs=8) as wrk,
            tc.tile_pool(name="sgp", bufs=3) as sgp,
            tc.tile_pool(name="fin", bufs=2) as fin,
            tc.tile_pool(name="psa", bufs=3, space="PSUM") as psa,
            tc.tile_pool(name="psb", bufs=3, space="PSUM") as psb,
            tc.tile_pool(name="dram", bufs=1, space="DRAM") as dramp,
        ):
            z_blk = [
                dramp.tile(
                    [128, TB[s_], 2 * D], f16, tag=f"zblk{s_}", name=f"zblk{s_}"
                )
                for s_ in range(NPB)
            ]

            # ---- constants ----
            WI_sb = constp.tile([128, 2, 2 * D], bf16)
            nc.sync.dma_start(WI_sb[:, :, :], WI_d[:, :, :])
            idx_sb = constp.tile([128, NIX // 16], mybir.dt.int16)
            nc.sync.dma_start(idx_sb[:, :], idx_d[:, :])

            # ---- phase A: Z = [exp(sh) | exp(sh)*h], sh = h @ W_self.T ----
            # one bf16 product (~0.4% on exp, averaged down by the softmax);
            # h reaches PSUM fp32 via the [W.T | I] identity columns.
            zq = None
            for g in range(NT_ALL // SLAB):
                hT_sb = pha.tile([128, SLAB, 2, 128], bf16, tag="hT")
                nc.sync.dma_start(hT_sb[:, :, :, :], hT_d[:, g, :, :, :])
                for j in range(SLAB):
                    i = g * SLAB + j
                    ps = psa.tile([128, 2 * D], f32, tag="ps")
                    for kb in range(2):
                        nc.tensor.matmul(
                            ps[:, :], hT_sb[:, j, kb, :], WI_sb[:, kb, :],
                            start=(kb == 0), stop=(kb == 1),
                        )
                    if i % WB == 0:
                        zq = phz.tile([128, WB, 2 * D], f16, tag="zq")
                    w = i % WB
                    e32 = wrk.tile([128, D], f32, tag="e32")
                    nc.scalar.activation(
                        e32[:, :], ps[:, 0:D], mybir.ActivationFunctionType.Exp
                    )
                    nc.vector.tensor_tensor(
                        zq[:, w, D:2 * D], e32[:, :], ps[:, D:2 * D],
                        mybir.AluOpType.mult,
                    )
                    nc.scalar.copy(zq[:, w, 0:D], e32[:, :])
                    if w == WB - 1:
                        s_ = 0 if i < TB[0] else 1
                        tb = i - TS[s_]
                        nc.sync.dma_start(
                            z_blk[s_][:, tb - (WB - 1):tb + 1, :], zq[:, :, :]
                        )

            # ---- phase B: per node-tile gather + segment softmax-sum ----
            # blk1 is only ready once phase A finishes; issue DELTA tiles of
            # blk0 gathers ahead so the in-order gpsimd queue stays busy
            # instead of stalling behind the first blk1 gather.
            offs = []
            o = 0
            for t in range(NT):
                offs.append([o, o + caps[t][0]])
                o += caps[t][0] + caps[t][1]
            C0M = max(c for c, _ in caps)
            C1M = max(c for _, c in caps)

            zx0_t = {}

            def issue_gather(t, s_, dest):
                Cs = caps[t][s_]
                if Cs == 0:
                    return
                CAPs = 128 * Cs
                io = offs[t][s_] * 8
                nc.gpsimd.dma_gather(
                    dest[:, 0:Cs, :],
                    z_blk[s_][:, :, :].flatten_outer_dims(),
                    idx_sb[:, io:io + 8 * Cs], CAPs, CAPs, 2 * D,
                    single_packet=False,
                )

            for t in range(DELTA):
                zx0_t[t] = gat.tile([128, C0M, 2 * D], f16, tag="zx0", name="zx0")
                issue_gather(t, 0, zx0_t[t])
            for t in range(NT):
                if t + DELTA < NT:
                    zx0_t[t + DELTA] = gat.tile([128, C0M, 2 * D], f16, tag="zx0", name="zx0")
                    issue_gather(t + DELTA, 0, zx0_t[t + DELTA])
                zx1 = gat1.tile([128, C1M, 2 * D], f16, tag="zx1")
                issue_gather(t, 1, zx1)
                c0 = caps[t][0]
                C_t = caps[t][0] + caps[t][1]
                Sg = sgp.tile([128, CMAX, 128], f16, tag="Sg")
                nc.sync.dma_start(
                    Sg[:, 0:C_t, :], S_d[:, offs[t][0]:offs[t][0] + C_t, :]
                )
                acc = psb.tile([128, 2 * D], f32, tag="acc")
                for j in range(C_t):
                    rhs = (zx0_t[t][:, j, :] if j < c0
                           else zx1[:, j - c0, :])
                    nc.tensor.matmul(
                        acc[:, :], Sg[:, j, :], rhs,
                        start=(j == 0), stop=(j == C_t - 1),
                    )
                zx0_t.pop(t)

                # ---- finalize tile: out = numer/denom, h for empty nodes ----
                dmax = fin.tile([128, D], f32, tag="dmax")
                nc.vector.tensor_scalar(
                    dmax[:, :], acc[:, 0:D], 1e-37, None, mybir.AluOpType.max
                )
                rec = fin.tile([128, D], f32, tag="rec")
                nc.vector.reciprocal(rec[:, :], dmax[:, :])
                res = fin.tile([128, D], f32, tag="res")
                nc.vector.tensor_tensor(
                    res[:, :], acc[:, D:2 * D], rec[:, :], mybir.AluOpType.mult
                )
                mask = fin.tile([128, D], mybir.dt.uint8, tag="mask")
                nc.vector.tensor_scalar(
                    mask[:, :], acc[:, 0:D], 0.0, None, mybir.AluOpType.is_equal
                )
                hown_sb = fin.tile([128, D], f32, tag="hown")
                nc.sync.dma_start(hown_sb[:, :], hown_d[t * 128:(t + 1) * 128, :])
                nc.vector.copy_predicated(res[:, :], mask[:, :], hown_sb[:, :])
                nc.sync.dma_start(out_d[t * 128:(t + 1) * 128, :], res[:, :])
    nc.compile()
    return nc


def _wrap_idx(ix):
    # dma_gather index layout: logical index i lands at output
    # [partition i%128, slot i//128]; the SBUF index tile stores it at
    # [i%16, 8*(i//128) + (i%128)//16], replicated over the 8 Q7 cores.
    w = ix.astype(np.int16).reshape(-1, 8, 16).transpose(2, 0, 1).reshape(16, -1)
    return np.tile(w, (8, 1))


def _balance_nodes(deg_blk, limits):
    """Assign nodes to CORES*NT bins (<=128 nodes each, occupancy free) so
    per-(bin, src-block) edge counts stay <= limits[s].  deg_blk:
    [N_NODES, NPB] in-degree split by src block.  Returns assign[node] = bin."""
    nbins = CORES * NT
    lim = np.asarray(limits, dtype=np.float64)
    order = np.argsort(-deg_blk.sum(axis=1), kind="stable")
    fill = np.zeros((nbins, NPB), dtype=np.int64)
    count = np.zeros(nbins, dtype=np.int64)
    assign = np.empty(N_NODES, dtype=np.int64)
    pos = 0
    while pos < N_NODES:
        # wave greedy: next wave of heavy nodes onto the least-loaded bins
        wave = order[pos:pos + nbins]
        avail = np.nonzero(count < 128)[0]
        ranked = avail[np.argsort(
            (fill[avail] / lim).max(axis=1) * 128 + count[avail] / 128.0,
            kind="stable")]
        k = min(len(wave), len(ranked))
        assign[wave[:k]] = ranked[:k]
        np.add.at(fill, (ranked[:k],), deg_blk[wave[:k]])
        np.add.at(count, ranked[:k], 1)
        pos += k
    # move repair: shift single nodes out of overfull (bin, block) cells
    for _ in range(4000):
        rel = fill / lim
        b = int(np.argmax(rel.max(axis=1)))
        if (fill[b] <= lim).all():
            break
        s = int(np.argmax(rel[b]))
        over = fill[b, s] - lim[s]
        nodes_b = np.nonzero(assign == b)[0]
        db = deg_blk[nodes_b]
        room = (count < 128) & (np.arange(nbins) != b)
        tgt = np.nonzero(room)[0]
        if len(tgt) == 0:
            break
        # smallest node that clears the overshoot in one move (else biggest)
        ds = db[:, s]
        clr = np.nonzero(ds >= over)[0]
        ni = (clr[np.argmin(db[clr].sum(axis=1))] if len(clr)
              else int(np.argmax(ds)))
        n = nodes_b[ni]
        ok = tgt[((fill[tgt] + deg_blk[n]) <= lim[None, :]).all(axis=1)]
        if len(ok) == 0:
            break
        dest = ok[np.argmin((fill[ok] / lim).max(axis=1) * 256 + count[ok])]
        fill[b] -= deg_blk[n]
        fill[dest] += deg_blk[n]
        count[b] -= 1
        count[dest] += 1
        assign[n] = dest
    return assign


def kernel(h, W_nb, b_nb, W_self, b_self, src, dst):
    from concourse.bass_utils import run_bass_kernel_spmd

    h = np.ascontiguousarray(np.asarray(h, dtype=np.float32))
    W = np.asarray(W_self, dtype=np.float32)
    src = np.asarray(src, dtype=np.int64)
    dst = np.asarray(dst, dtype=np.int64)

    # partition-interleaved Z row of each node (original id space; phase A
    # and the gather side are unaffected by the dst rebalancing permutation)
    tg = src // 128
    zblk_e = (tg >= TB[0]).astype(np.int64)
    zrow_e = (src % 128) * np.where(zblk_e == 0, TB[0], TB[1]) + (
        tg - np.where(zblk_e == 0, TS[0], TS[1])
    )

    # --- rebalance dst nodes across (core, tile) bins ---
    # node -> row (bin*128 + slot) with free per-bin occupancy; empty rows
    # get denom 0 on device and are dropped by the host unshard.
    deg_blk = np.zeros((N_NODES, NPB), dtype=np.int64)
    np.add.at(deg_blk, (dst, zblk_e), 1)
    assign = _balance_nodes(deg_blk, LIMITS)  # node -> bin
    o_bin = np.argsort(assign, kind="stable")
    slot = np.arange(N_NODES) - np.searchsorted(
        assign[o_bin], assign[o_bin], side="left"
    )
    noderow = np.empty(N_NODES, dtype=np.int64)
    noderow[o_bin] = assign[o_bin] * 128 + slot          # node -> row
    rownode = np.full(CORES * NROWS, -1, dtype=np.int64)
    rownode[noderow] = np.arange(N_NODES)                # row -> node | -1
    dstb = noderow[dst]                                  # balanced dst rows

    order = np.argsort(dstb, kind="stable")
    src_s = src[order]
    dstb_s = dstb[order]
    zrow_s = zrow_e[order]
    zblk_s = zblk_e[order]

    # per-(core, tile) edge ranges; tiles are 128 consecutive balanced rows
    tile_base = np.arange(CORES * NT) * 128
    bounds_lo = np.searchsorted(dstb_s, tile_base, side="left")
    bounds_hi = np.searchsorted(dstb_s, tile_base + 128, side="left")

    # split each tile's edges by src block; caps shared across cores (SPMD)
    per_ct = {}
    cnt = np.zeros((CORES, NT, NPB), dtype=np.int64)
    for c in range(CORES):
        for t in range(NT):
            i = c * NT + t
            lo, hi = int(bounds_lo[i]), int(bounds_hi[i])
            blk = zblk_s[lo:hi]
            for s_ in range(NPB):
                sel = np.nonzero(blk == s_)[0]
                zr = zrow_s[lo:hi][sel]
                dl = dstb_s[lo:hi][sel] - tile_base[i]
                o2 = np.argsort(zr, kind="stable")   # ascending gather addrs
                per_ct[(c, t, s_)] = (zr[o2], dl[o2])
                cnt[c, t, s_] = len(sel)
    caps = [
        [int((cnt[:, t, s_].max() + 127) // 128) for s_ in range(NPB)]
        for t in range(NT)
    ]
    assert max(a + b for a, b in caps) <= 40, f"edge distribution too skewed: {caps}"
    NCH = sum(sum(r) for r in caps)

    # host-side layout prep
    import ml_dtypes
    bf = ml_dtypes.bfloat16
    h_pad = np.zeros((NPAD, D), dtype=np.float32)
    h_pad[:N_NODES] = h
    hT = np.ascontiguousarray(
        h_pad.astype(bf).T.reshape(2, 128, NT_ALL // SLAB, SLAB, 128)
        .transpose(1, 2, 3, 0, 4)
    )
    WIfull = np.zeros((D, 2 * D), dtype=np.float32)
    WIfull[:, :D] = W.T
    WIfull[np.arange(D), D + np.arange(D)] = 1.0
    WI = np.ascontiguousarray(
        WIfull.reshape(2, 128, 2 * D).transpose(1, 0, 2).astype(bf)
    )

    in_maps = []
    for c in range(CORES):
        idx_parts = []
        S_all = np.zeros((128, NCH, 128), dtype=np.float16)
        coff = 0
        for t in range(NT):
            for s_ in range(NPB):
                Cs = caps[t][s_]
                if Cs == 0:
                    continue
                CAPs = 128 * Cs
                zr, dl = per_ct[(c, t, s_)]
                n = len(zr)
                zpad = np.zeros(CAPs, dtype=np.int64)
                zpad[:n] = zr
                idx_parts.append(_wrap_idx(zpad))
                ei = np.arange(n)
                S_all[ei % 128, coff + ei // 128, dl] = 1.0
                coff += Cs
        rn = rownode[c * NROWS:(c + 1) * NROWS]
        hown = np.zeros((NROWS, D), dtype=np.float32)
        hown[rn >= 0] = h[rn[rn >= 0]]
        in_maps.append({
            "hT": hT,
            "WI": WI,
            "S": S_all,
            "idx": np.ascontiguousarray(np.concatenate(idx_parts, axis=1)),
            "hown": hown,
        })

    key = tuple(tuple(r) for r in caps)
    if key not in _cache:
        _cache[key] = _build(caps)
    nc = _cache[key]

    res = run_bass_kernel_spmd(nc, in_maps, core_ids=list(range(CORES)))
    outb = np.concatenate(
        [res.results[c]["out"] for c in range(CORES)], axis=0
    )
    out = np.empty((N_NODES, D), dtype=np.float32)
    valid = rownode >= 0
    out[rownode[valid]] = outb[valid]         # un-permute balanced rows
    return out.astype(np.float32)
